# revision 45
# baseline (speedup 1.0000x reference)
"""Causal MHA forward on 8 NeuronCores (Trainium2, Bass/Tile) — v8.

Sharding: batch (4) x head-half (2) -> 8 cores; each core handles one batch
and 8 heads (a 512-wide column slice of the QKV projections and the matching
512-row slice of dense_w). Host sums the two partial dense outputs per batch
and adds dense_b + wv_b @ dense_w.

Compute strategy (same math as v5):
- Q/K projections in fp8e4 DoubleRow (x and 16*w as e4m3); V projection and
  dense stay bf16. Scores in fp8 DoubleRow with the d=64 contraction split
  32+32 into the two DoubleRow k-tiles. Diagonal superblocks compute only
  live q-columns; causal triangle zeroed in pt by GPSIMD (DVE for the tail
  pair, where Pool is the serializer). exp on Act (bf16 out). PV in q-major
  bf16 with a ones column for the rowsum; normalization = per-partition
  reciprocal (DVE) + scalar multiply (GPSIMD). O^T via DMA XBAR transpose,
  except the last pair which uses a PE matmul-transpose (recycled fill psum
  bank bitcast to bf16) to cut tail latency. Dense bf16.

Scheduling (v8; Act engine is the roofline at ~152us busy, so every rule
below protects the exp cadence):
- Score-ahead emission: slot t+1's score matmuls are emitted right after
  exp(t) and BEFORE PV/fill pumping, so on the in-order PE queue the next
  exp's input never waits behind a fill burst.
- Gentle pump budgets (~0.3x of the exp window, floor 250 rows): stations
  are mostly pulled just-in-time by the prefix gates (bursts land between
  score emissions, which is Act-safe under score-ahead), and over-pumping
  only feeds the cross-engine counter-sync convoys.
- Station split: K/Q g0 of chunk c enqueued at chunk c-1 start; V and K/Q
  g1 at chunk c's own start (chunk 0: g1 before V, since its V data is
  last on the serialized DMA stream; prefix gates renumbered to match).
- Per-chunk PV lag: chunk 0 lags 32 slots (pt pool 36-deep) so its PV/norm
  backlog drains across chunk 1 after the late V data lands; later chunks
  lag 5. PV emission is a monotonic pointer so mixed lags stay ordered.
- Tri-zeroing on DVE (bf16 SBUF->SBUF 4x mode, ~127ns): the diagonal PV
  Ldweights waits on it, and DVE never blocks on the norm-transpose chain
  the way GPSIMD/Pool does (that coupling was a 5us convoy).
- Startup: weights arrive as four host-prearranged contiguous pieces
  (strided halves pay a 2x DMA <512B latency multiplier), x fp8 quarters
  early, x8[1] before wv/xb, wd after xb[1]; first exp at ~10.2us.
- Tail: dense of the last chunk activates per-slab in the drain with an
  unthrottled pump, allocates its psum from the (by then idle) sc rotation
  so stations pipeline instead of ping-ponging on the 2-deep fill tag, its
  psum->sbuf copies run on the (idle) Act engine, the last pair's O^T
  transposes use a PE matmul-transpose into a recycled fill psum bank
  (bitcast to bf16), and output DMAs go per-sb (last sb per half).
"""
import numpy as np
import ml_dtypes

import concourse.bacc as bacc
import concourse.bass as bass
import concourse.tile as tile
import concourse.mybir as mybir
from concourse.bass_utils import run_bass_kernel_spmd

B, S, D = 4, 2048, 1024
DC = 512           # per-core d slice (8 heads x 64)
H = 8              # heads per core
DH = 64
N_CORES = 8
F32 = mybir.dt.float32
BF = mybir.dt.bfloat16
F8 = mybir.dt.float8e4
AF = mybir.ActivationFunctionType
DRM = mybir.MatmulPerfMode.DoubleRow
WS = 16.0                       # host-side Q/K weight prescale (fp8 range)
SCALE = 1.0 / 32.0 / (WS * WS)  # exp scale: 1/sqrt(D_MODEL) / WS^2
_CACHE = {}


def _build():
    nc = bacc.Bacc("TRN2", target_bir_lowering=False, debug=False,
                   num_devices=N_CORES)
    xt8 = nc.dram_tensor("xt8", [D, S], F8, kind="ExternalInput")
    xtb = nc.dram_tensor("xtb", [D, S], BF, kind="ExternalInput")
    # K h0 | Q h0 | K h1 | Q h1, each host-prearranged as [128, 8, 256]
    # (partition, c-block, station cols) so every DMA is fully contiguous
    wkq = nc.dram_tensor("wkq", [4, 128, 8, 256], F8, kind="ExternalInput")
    wv = nc.dram_tensor("wv", [D, DC], BF, kind="ExternalInput")
    qb = nc.dram_tensor("qb", [128, 4], F32, kind="ExternalInput")
    kb = nc.dram_tensor("kb", [128, 4], F32, kind="ExternalInput")
    wd = nc.dram_tensor("wd", [DC, D], BF, kind="ExternalInput")
    tri = nc.dram_tensor("tri", [128, 2, 128], BF, kind="ExternalInput")
    eye = nc.dram_tensor("eye", [128, 128], BF, kind="ExternalInput")
    out = nc.dram_tensor("out", [S, D], BF, kind="ExternalOutput")

    with tile.TileContext(nc) as tc:
      with nc.allow_low_precision(reason="fp8/bf16 compute; all matmul accumulation in fp32 psum"):
        with (
            tc.tile_pool(name="consts", bufs=1) as consts,
            tc.tile_pool(name="wgt", bufs=1) as wgt,
            tc.tile_pool(name="big", bufs=1) as big,
            tc.tile_pool(name="qtp", bufs=2) as qtp,
            tc.tile_pool(name="x8p", bufs=6) as x8p,
            tc.tile_pool(name="xbp", bufs=6) as xbp,
            tc.tile_pool(name="ptp", bufs=36) as ptp,
            tc.tile_pool(name="pvsp", bufs=2) as pvsp,
            tc.tile_pool(name="obp", bufs=3) as obp,
            tc.tile_pool(name="rsp", bufs=2) as rsp,
            tc.tile_pool(name="osp", bufs=2) as osp,
            tc.tile_pool(name="scp", bufs=2, space="PSUM") as scp,
            tc.tile_pool(name="pvp", bufs=1, space="PSUM") as pvp,
        ):
            xv8 = xt8.ap().rearrange("(i p) s -> p i s", p=128)
            xvb = xtb.ap().rearrange("(i p) s -> p i s", p=128)
            xg8s, xgbs = {}, {}

            def load_x8(c):
                a8 = x8p.tile([128, 4, 512], F8, tag="x8", name=f"x8a{c}")
                b8 = x8p.tile([128, 4, 512], F8, tag="x8", name=f"x8b{c}")
                nc.sync.dma_start(out=a8, in_=xv8[:, 0:4, 512 * c:512 * (c + 1)])
                nc.sync.dma_start(out=b8, in_=xv8[:, 4:8, 512 * c:512 * (c + 1)])
                xg8s[c] = [a8, b8]

            def load_xb(c):
                ab = xbp.tile([128, 4, 512], BF, tag="xb", name=f"xba{c}")
                bb = xbp.tile([128, 4, 512], BF, tag="xb", name=f"xbb{c}")
                nc.sync.dma_start(out=ab, in_=xvb[:, 0:4, 512 * c:512 * (c + 1)])
                nc.sync.dma_start(out=bb, in_=xvb[:, 4:8, 512 * c:512 * (c + 1)])
                xgbs[c] = [ab, bb]

            # ---- startup DMAs: one shared serialized DMA stream, ordered
            # by first-use; the K/Q weight pieces are host-contiguous ----
            wk_t = [wgt.tile([128, 8, 256], F8, name=f"wk{h}") for h in range(2)]
            wq_t = [wgt.tile([128, 8, 256], F8, name=f"wq{h}") for h in range(2)]
            a8 = x8p.tile([128, 4, 512], F8, tag="x8", name="x8a0")
            b8 = x8p.tile([128, 4, 512], F8, tag="x8", name="x8b0")
            xg8s[0] = [a8, b8]
            nc.sync.dma_start(out=wk_t[0], in_=wkq.ap()[0])
            nc.sync.dma_start(out=a8[:, 0:2, :], in_=xv8[:, 0:2, 0:512])
            nc.sync.dma_start(out=a8[:, 2:4, :], in_=xv8[:, 2:4, 0:512])
            nc.sync.dma_start(out=b8[:, 0:2, :], in_=xv8[:, 4:6, 0:512])
            nc.sync.dma_start(out=b8[:, 2:4, :], in_=xv8[:, 6:8, 0:512])
            nc.sync.dma_start(out=wq_t[0], in_=wkq.ap()[1])
            kb_sb = consts.tile([128, 4], F32)
            nc.sync.dma_start(out=kb_sb, in_=kb.ap())
            qb_sb = consts.tile([128, 4], F32)
            nc.sync.dma_start(out=qb_sb, in_=qb.ap())
            tri_sb = consts.tile([128, 2, 128], BF)
            nc.sync.dma_start(out=tri_sb, in_=tri.ap())
            nc.sync.dma_start(out=wk_t[1], in_=wkq.ap()[2])
            nc.sync.dma_start(out=wq_t[1], in_=wkq.ap()[3])
            load_x8(1)
            wv_sb = wgt.tile([128, 8, DC], BF)
            nc.sync.dma_start(out=wv_sb, in_=wv.ap().rearrange("(c p) d -> p c d", p=128))
            load_xb(0)
            load_xb(1)
            wd_sb = wgt.tile([128, 4, D], BF)
            nc.sync.dma_start(out=wd_sb, in_=wd.ap().rearrange("(c p) d -> p c d", p=128))
            eye_sb = consts.tile([128, 128], BF)
            nc.sync.dma_start(out=eye_sb, in_=eye.ap())

            def load_xts(c):
                load_x8(c)
                load_xb(c)

            # ---- persistent big tensors ----
            kt = big.tile([128, 2, 2, S], F8)
            va = big.tile([128, 16, H, 65], BF)   # [k-in-block, kblock, head, d|1]
            ot = big.tile([128, 4, S], BF)        # O^T: part = d of head pair
            nc.vector.memset(va[:, :, :, 64:65], 1.0)

            qts = {}

            # ---- fill stations (generators yielding approx PE rows/step) ----
            def st_kq(cc, g, dh, which):
                w_t, bias, dest = ((wk_t, kb_sb, kt) if which == "k"
                                   else (wq_t, qb_sb, qts[cc]))
                xg = xg8s[cc]
                ps = scp.tile([128, 512], F32, tag="fill", bufs=2,
                              name=f"f{which}{cc}_{g}{dh}")
                s = 2 * g + dh  # host pre-permuted w columns: contiguous slice
                w_sb = w_t[s // 2]
                sl = 128 * (s % 2)
                for m in range(4):
                    nc.tensor.matmul(ps, w_sb[:, 2 * m:2 * m + 2, sl:sl + 128],
                                     xg[m // 2][:, 2 * (m % 2):2 * (m % 2) + 2, :],
                                     start=(m == 0), stop=(m == 3), perf_mode=DRM)
                    yield 256
                col = 2 * g + dh
                if which == "k":
                    o = dest[:, g, dh, 512 * cc:512 * (cc + 1)]
                else:
                    o = dest[:, g, dh, :]
                nc.vector.tensor_scalar_add(out=o, in0=ps,
                                            scalar1=bias[:, col:col + 1])
                yield 64

            def st_v(cc, sb):
                xg = xgbs[cc]
                ps = scp.tile([128, 512], F32, tag="fill", bufs=2, name=f"fv{cc}_{sb}")
                for i in range(8):
                    nc.tensor.matmul(ps, xg[i // 4][:, i % 4, 128 * sb:128 * (sb + 1)],
                                     wv_sb[:, i, :], start=(i == 0), stop=(i == 7))
                    yield 512
                nc.vector.tensor_copy(
                    out=va[:, 4 * cc + sb, :, 0:64],
                    in_=ps.rearrange("p (h d) -> p h d", h=H))
                yield 64

            os4 = {}

            def st_dense(cc, sb):
                if cc not in os4:
                    os4[cc] = osp.tile([128, 4, 1024], BF, tag="os", name=f"os{cc}")
                os_t = os4[cc]
                ov = out.ap().rearrange("(sb p) d -> p sb d", p=128)
                for n in range(2):
                    if cc == 3:
                        # drain-time: the exp pipeline is done, so the sc
                        # banks are free — 2 extra psum tiles let the dense
                        # stations pipeline instead of ping-ponging on the
                        # 2-deep fill rotation
                        ps = scp.tile([128, 2, 512], F32, tag="sc",
                                      name=f"fd{cc}_{sb}_{n}")[:, 0, :]
                    else:
                        ps = scp.tile([128, 512], F32, tag="fill", bufs=2, name=f"fd{cc}_{sb}_{n}")
                    for pp in range(4):
                        nc.tensor.matmul(ps, ot[:, pp, 128 * sb:128 * (sb + 1)],
                                         wd_sb[:, pp, 512 * n:512 * (n + 1)],
                                         start=(pp == 0), stop=(pp == 3))
                        yield 512
                    if cc == 3:
                        nc.scalar.copy(out=os_t[:, sb % 4, 512 * n:512 * (n + 1)], in_=ps)
                    else:
                        nc.vector.tensor_copy(out=os_t[:, sb % 4, 512 * n:512 * (n + 1)], in_=ps)
                    yield 64
                    if cc == 3 and sb == 15:
                        # overlap the very last copy with its own DMA
                        nc.sync.dma_start(
                            out=ov[:, sb:sb + 1, 512 * n:512 * (n + 1)],
                            in_=os_t[:, 3:4, 512 * n:512 * (n + 1)])
                        yield 16
                if cc == 3 and sb != 15:
                    nc.sync.dma_start(out=ov[:, sb:sb + 1, :],
                                      in_=os_t[:, sb % 4:sb % 4 + 1, :])
                    yield 16
                elif cc != 3 and sb % 4 == 3:
                    nc.sync.dma_start(out=ov[:, 4 * cc:4 * cc + 4, :], in_=os_t)
                    yield 16

            dense_q, station_q = [], []
            station_done = {}

            def _step_station():
                c0, g0 = station_q[0]
                try:
                    return next(g0)
                except StopIteration:
                    station_q.pop(0)
                    station_done[c0] = station_done.get(c0, 0) + 1
                    return 0

            def pump(budget):
                while budget > 0 and (dense_q or station_q):
                    if station_q:
                        budget -= _step_station()
                    else:
                        g0 = dense_q.pop(0)
                        while True:
                            try:
                                budget -= next(g0)
                            except StopIteration:
                                break

            def gate(c, n):
                while station_done.get(c, 0) < n and station_q:
                    _step_station()

            def drain_all():
                pump(float("inf"))
                while station_q:
                    _step_station()

            # ---- attention machinery ----
            pts = {}

            def scores_only(c, p, j):
                qt = qts[c]
                sc = scp.tile([128, 2, 512], F32, tag="sc", name=f"sc{c}_{p}_{j}")
                pt_t = ptp.tile([128, 2, 512], BF, tag="pt", name=f"pt{c}_{p}_{j}")
                pts[(c, p, j)] = (sc, pt_t)
                jj = j - 4 * c
                base = 128 * jj if jj >= 0 else 0
                for hh in range(2):
                    h = 2 * p + hh
                    g, r = h // 4, h % 4
                    nc.tensor.matmul(sc[:, hh, base:512],
                                     kt[32 * r:32 * (r + 1), g, :, 128 * j:128 * (j + 1)],
                                     qt[32 * r:32 * (r + 1), g, :, base:512],
                                     start=True, stop=True, perf_mode=DRM,
                                     tile_position=(32 * r, 0))

            def exp_only(c, p, j):
                sc, pt_t = pts[(c, p, j)]
                jj = j - 4 * c
                base = 128 * jj if jj >= 0 else 0
                nc.scalar.activation(out=pt_t[:, :, base:512],
                                     in_=sc[:, :, base:512], func=AF.Exp,
                                     scale=SCALE)
                if jj >= 0:
                    # zero the causal triangle of the diagonal 128-col slab.
                    # On DVE (bf16 SBUF->SBUF 4x mode, ~127ns): the diag PV's
                    # Ldweights waits on this, and DVE never blocks on the
                    # norm-transpose chain the way Pool does.
                    nc.vector.tensor_tensor(
                        out=pt_t[:, :, base:base + 128],
                        in0=pt_t[:, :, base:base + 128],
                        in1=tri_sb, op=mybir.AluOpType.mult)

            pv_cur = {}

            def pv_contrib(c, p, j):
                if (c, p) not in pv_cur:
                    pv_cur[(c, p)] = (pvp.tile([128, 4, 2, 128], F32, tag="pv",
                                               name=f"pv{c}_{p}"), set())
                pv, started = pv_cur[(c, p)]
                jj = j - 4 * c  # >= 0 only for diagonal blocks
                pt_t = pts[(c, p, j)][1]
                for qq in range(max(jj, 0), 4):
                    for hh in range(2):
                        bank = qq // 2
                        start = bank not in started
                        started.add(bank)
                        stop = (hh == 1 and qq % 2 == 1 and j == 4 * c + qq)
                        nc.tensor.matmul(pv[:, qq, hh, 0:65],
                                         pt_t[:, hh, 128 * qq:128 * (qq + 1)],
                                         va[:, j, 2 * p + hh, :],
                                         start=start, stop=stop,
                                         skip_group_check=True)

            ob_cur = {}

            def norm_xbar_slab(c, p, qq):
                pv, _ = pv_cur[(c, p)]
                if (c, p) not in ob_cur:
                    ob_cur[(c, p)] = obp.tile([128, 4, 2, 64], BF, tag="ob",
                                              name=f"ob{c}_{p}")
                obt = ob_cur[(c, p)]
                pvs = pvsp.tile([128, 2, 65], F32, tag="pvs", bufs=6,
                                name=f"pvs{c}_{p}_{qq}")
                nc.vector.tensor_copy(out=pvs, in_=pv[:, qq, :, 0:65])
                rs = rsp.tile([128, 2, 1], F32, tag="rs", bufs=6,
                              name=f"rs{c}_{p}_{qq}")
                nc.vector.reciprocal(out=rs, in_=pvs[:, :, 64:65])
                for hh in range(2):
                    nc.gpsimd.tensor_scalar_mul(out=obt[:, qq, hh, :],
                                                in0=pvs[:, hh, 0:64],
                                                scalar1=rs[:, hh, :])
                if c == 3 and p == 3:
                    # tail path: PE transpose into a recycled fill psum bank
                    # (bitcast to bf16) -> DVE copy to ot; ~2us less latency
                    # than the DMA XBAR per slab
                    tp = scp.tile([128, 512], F32, tag="fill", bufs=2,
                                  name=f"tp{qq}")
                    tpb = tp[:, 0:64].bitcast(BF)
                    nc.tensor.transpose(out=tpb, in_=obt[:, qq, :, :],
                                        identity=eye_sb[:, :])
                    nc.vector.tensor_copy(
                        out=ot[:, p, 128 * (4 * c + qq):128 * (4 * c + qq + 1)],
                        in_=tpb)
                else:
                    nc.sync.dma_start(
                        out=ot[:, p, 128 * (4 * c + qq):128 * (4 * c + qq + 1)],
                        in_=obt[:, qq, :, :], transpose=True)
                if qq == 3:
                    pv_cur.pop((c, p))
                    ob_cur.pop((c, p))

            # pump budget per slot ~ the exp window minus the inline
            # scores/PV work emitted the same iteration
            def slot_budget(c, j):
                jj = j - 4 * c
                cols = 1024 - 256 * max(jj, 0)
                exp_ns = cols * 0.8333 + 185.0
                return max(int(0.80 * 2.4 * exp_ns) - 1130, 250)

            # ================= emission =================
            def enqueue_g0(cc):
                qts[cc] = qtp.tile([128, 2, 2, 512], F8, tag="qt",
                                   name=f"qt{cc}")
                for dhf in range(2):
                    station_q.append((cc, st_kq(cc, 0, dhf, "k")))
                    station_q.append((cc, st_kq(cc, 0, dhf, "q")))

            def enqueue_rest(cc):
                # g1 before V for every chunk: g1's inputs (x fp8 + weight
                # halves) always land early, while the bf16 x for V is late
                # on the serialized DMA stream — pumping V first head-of-line
                # blocks the PE queue on the xb DMA
                for dhf in range(2):
                    station_q.append((cc, st_kq(cc, 1, dhf, "k")))
                    station_q.append((cc, st_kq(cc, 1, dhf, "q")))
                for sbb in range(4):
                    station_q.append((cc, st_v(cc, sbb)))

            def g1_prefix(cc):
                return 8

            def v_prefix(cc, jj):
                return 9 + jj

            enqueue_g0(0)
            enqueue_rest(0)
            enqueue_g0(1)

            slots = [(c, p, j) for c in range(4) for p in range(4)
                     for j in range(4 * c + 4)]

            def lag_of(c, jj=-1):
                return 32 if c == 0 else 5


            dense_pending = []  # (activate_slot, generator)

            def emit_lagged(t):
                c, p, j = slots[t]
                jj = j - 4 * c
                if jj >= 0:
                    gate(c, v_prefix(c, jj))  # diag PV reads va block 4c+jj
                pv_contrib(c, p, j)
                if jj >= 0:
                    norm_xbar_slab(c, p, jj)
                    if p == 3:
                        # activate well after the norm DMA-transposes are
                        # emitted (~2.5us latency) so the dense matmuls never
                        # head-of-line block the in-order PE queue
                        act = t + (1 if c == 3 else lag_of(c, jj) + 7)
                        dense_pending.append((act, st_dense(c, 4 * c + jj)))

            next_pv = [0]

            def pump_pv(t):
                # emit PV/norm for every slot whose per-chunk lag has elapsed
                while (next_pv[0] < len(slots)
                       and t - next_pv[0] >= lag_of(
                           slots[next_pv[0]][0],
                           slots[next_pv[0]][2] - 4 * slots[next_pv[0]][0])):
                    emit_lagged(next_pv[0])
                    next_pv[0] += 1

            def gates_for(t):
                if 0 <= t < len(slots):
                    c2, p2, j2 = slots[t]
                    if j2 == 0:
                        gate(c2, 4 if p2 < 2 else g1_prefix(c2))

            def boundary_for(t):
                if t >= len(slots):
                    return
                c2, p2, j2 = slots[t]
                if p2 == 0 and j2 == 0 and c2 >= 1:
                    if c2 + 1 <= 3:
                        load_xts(c2 + 1)  # prologue already loaded 0 and 1
                    enqueue_rest(c2)
                    if c2 + 1 <= 3:
                        enqueue_g0(c2 + 1)

            gates_for(0)
            scores_only(*slots[0])
            for t, (c, p, j) in enumerate(slots):
                exp_only(c, p, j)
                boundary_for(t + 1)
                gates_for(t + 1)
                if t + 1 < len(slots):
                    scores_only(*slots[t + 1])
                gates_for(t + 2)
                while dense_pending and dense_pending[0][0] <= t:
                    dense_q.append(dense_pending.pop(0)[1])
                pump_pv(t)
                pump(float("inf") if t >= len(slots) - 6 else slot_budget(c, j))

            # drain: emit remaining PV/norm chains, pumping dense as soon as
            # each becomes ready so the last chunk's dense overlaps the norms
            t = len(slots)
            while next_pv[0] < len(slots):
                emit_lagged(next_pv[0])
                next_pv[0] += 1
                while dense_pending and dense_pending[0][0] <= next_pv[0] + 5:
                    dense_q.append(dense_pending.pop(0)[1])
                pump(float("inf"))
            for _, g in dense_pending:
                dense_q.append(g)
            dense_pending.clear()
            drain_all()

    nc.compile()
    return nc


def get_nc():
    if "nc" not in _CACHE:
        _CACHE["nc"] = _build()
    return _CACHE["nc"]


def _f8(a):
    return np.ascontiguousarray(a.astype(ml_dtypes.float8_e4m3fn))


def _bf(a):
    return np.ascontiguousarray(a.astype(ml_dtypes.bfloat16))


def _split_bias(bias):
    # [512] -> [128, 4]: partition 32*(h%4)+dl, col (g, dh)
    v = bias.reshape(2, 4, 2, 32)         # g, h, dh, dl
    return np.ascontiguousarray(
        v.transpose(1, 3, 0, 2).reshape(128, 4))


# column permutation so the kernel's contiguous 128-col station slices land
# in the [32-part x (head, dhalf)] score layout: new 128s+32h+dl takes old
# 256g+64h+32dh+dl with s = 2g+dh
_QK_PERM = np.array([256 * (s // 2) + 64 * h + 32 * (s % 2) + dl
                     for s in range(4) for h in range(4) for dl in range(32)])


def _wkq_pieces(wk_w, wq_w):
    # host-prearranged contiguous pieces [4, 128, 8, 256]:
    # piece i = (K half0, Q half0, K half1, Q half1); layout matches the
    # SBUF tile [partition, c-block, station-col-half]
    def piece(w, half):
        v = w[:, 256 * half:256 * (half + 1)]         # [1024, 256]
        v = v.reshape(8, 128, 256).transpose(1, 0, 2)  # [128, 8, 256]
        return v
    return np.ascontiguousarray(np.stack([
        piece(wk_w, 0), piece(wq_w, 0), piece(wk_w, 1), piece(wq_w, 1)]))


def kernel(x, mask, wq_w, wq_b, wk_w, wk_b, wv_w, wv_b, dense_w, dense_b,
           _trace=False):
    x = np.asarray(x, dtype=np.float32)
    wq_w = np.asarray(wq_w, np.float32); wq_b = np.asarray(wq_b, np.float32)
    wk_w = np.asarray(wk_w, np.float32); wk_b = np.asarray(wk_b, np.float32)
    wv_w = np.asarray(wv_w, np.float32); wv_b = np.asarray(wv_b, np.float32)
    dense_w = np.asarray(dense_w, np.float32)
    dense_b = np.asarray(dense_b, np.float32)

    # 0/1 multiplicative causal mask for a 128x128 diagonal block in
    # S^T[k, q] coords: zero (disallow) iff q < k; duplicated per head-half
    k_idx = np.arange(128)[:, None]
    q_idx = np.arange(128)[None, :]
    tri1 = np.where(q_idx < k_idx, 0.0, 1.0).astype(np.float32)
    tri = np.ascontiguousarray(np.broadcast_to(tri1[:, None, :], (128, 2, 128)))
    eye = np.eye(128, dtype=np.float32)

    in_maps = []
    for core in range(N_CORES):
        b, hh = divmod(core, 2)
        sl = slice(DC * hh, DC * (hh + 1))
        in_maps.append({
            "xt8": _f8(x[b].T),
            "xtb": _bf(x[b].T),
            "wkq": _f8(_wkq_pieces(wk_w[:, sl][:, _QK_PERM] * WS,
                                   wq_w[:, sl][:, _QK_PERM] * WS)),
            "wv": _bf(wv_w[:, sl]),
            "qb": _split_bias(wq_b[sl] * WS),
            "kb": _split_bias(wk_b[sl] * WS),
            "wd": _bf(dense_w[sl, :]),
            "tri": _bf(tri),
            "eye": _bf(eye),
        })
    nc = get_nc()
    res = run_bass_kernel_spmd(nc, in_maps, core_ids=list(range(N_CORES)),
                               trace=_trace)
    const = dense_b + wv_b @ dense_w  # bias terms deferred to host
    outs = np.empty((B, S, D), np.float32)
    for b in range(B):
        outs[b] = (np.asarray(res.results[2 * b]["out"], dtype=np.float32)
                   + np.asarray(res.results[2 * b + 1]["out"], dtype=np.float32)
                   + const)
    if _trace:
        kernel.last_result = res
    return outs
